# revision 1
# baseline (speedup 1.0000x reference)
"""Trainium2 Bass kernel for CombinedVectorField (CFG vector field + exact
Jacobian-trace divergence).

Math: with u = tanh(x@W1x + h@W1h + b1'), b1' = b1 + t*W1[256],
  v(x,h)  = u @ W2 + b2
  div(x,h)= sum_k (1-u_k^2) c_k = d0 - (u*u) @ c,   c_k = sum_i W1x[i,k] W2[k,i]
Output = concat[(1-gs)*v_null + gs*v_h, (1-gs)*div_null + gs*div_h].

Sharding: pure data parallel — each of the 8 cores takes 512 batch rows
(both guidance branches), weights replicated. All tensors are kept
feature-major (transposed) on device so every matmul contracts over the
partition dim; host does the transposes/reshapes only.
"""
import sys

sys.path.insert(0, "/opt/trn_rl_repo")

import ml_dtypes
import numpy as np

import concourse.bass as bass
import concourse.tile as tile
from concourse import bacc, mybir
from concourse.bass_utils import run_bass_kernel_spmd
from concourse.vector_clock import ScopedClock


class _TrimTileContext(tile.TileContext):
    """TileContext with the final all-engine barrier dropped from the
    teardown and the mid barrier reduced to sem-only (no per-engine
    drains). The head drain still waits for every semaphore (incl.
    output-DMA completion) and semaphores are still cleared for the next
    execution; only the trailing barrier (nothing executes after it) is
    elided."""

    def _drain_and_barrier(self, tick_clock, wait_clock):
        drain_inst = self.nc.sync.drain()
        wait_clock.add_sem_waits(
            drain_inst.ins, ScopedClock({None: tick_clock.global_clock})
        )
        self.nc.all_engine_barrier(sem_only=True)
        popped = self.nc._tile_sem_poison_stack.pop()
        assert popped is self._sem_poison
        self.nc.clear_and_free_semaphores(list(self.sems.allocated().values()))


class _FastBacc(bacc.Bacc):
    """Bacc whose constructor-time all-engine barrier (after the const-tile
    memsets) is sem-only — the per-engine drains there cost ~1us of kernel
    head time and order nothing we rely on beyond the memsets, which the
    event-semaphore barrier already orders."""

    def all_engine_barrier(self, *, sem_only: bool = False):
        super().all_engine_barrier(sem_only=True)

F32 = mybir.dt.float32
BF16 = mybir.dt.bfloat16
AF = mybir.ActivationFunctionType
ALU = mybir.AluOpType

N_CORES = 8
B = 4096
DIM_X = 128
DIM_H = 128
HIDDEN = 512
R = B // N_CORES          # rows per core
NCH = HIDDEN // 128       # hidden chunks
W2W = NCH * DIM_X + NCH   # w2 chunks + cmat columns

_NC_CACHE = None


def _build():
    nc = _FastBacc("TRN2", target_bir_lowering=False, debug=False,
                   enable_asserts=False, monotonic_sem_count=0)

    # four bf16 input blobs, alternating over the two HWDGE rings so the
    # first-matmul gate (A1 + B1) is as small as possible; the w2 blob (B2)
    # is only needed once the first tanh completes.
    #   A1 = [xT | w1x]   A2 = [w1h]   B1 = [hT | hnT]
    #   B2 = [gs*w2r | (1-gs)*w2r | -gs*cmat | -(1-gs)*cmat]
    # (guidance-scale combine folded into the weights on the host, so PSUM
    #  accumulates the already-combined v and div directly)
    inA1 = nc.dram_tensor("inA1", [128, R + HIDDEN], BF16, kind="ExternalInput")
    inA2 = nc.dram_tensor("inA2", [128, HIDDEN], BF16, kind="ExternalInput")
    inB1 = nc.dram_tensor("inB1", [128, 2 * R], BF16, kind="ExternalInput")
    inB2 = nc.dram_tensor("inB2", [128, 2 * W2W], BF16, kind="ExternalInput")
    # aux cols: 0-3 b1' chunks, 4 b2, 5 d0
    aux = nc.dram_tensor("aux", [128, 6], F32, kind="ExternalInput")

    VO = nc.dram_tensor("VO", [DIM_X, R], F32, kind="ExternalOutput")
    DO = nc.dram_tensor("DO", [1, R], F32, kind="ExternalOutput")

    with _TrimTileContext(nc) as tc:
        with tc.tile_pool(name="cst", bufs=1) as cst, \
             tc.tile_pool(name="act", bufs=3) as actp, \
             tc.tile_pool(name="out", bufs=1) as outp, \
             tc.tile_pool(name="psa", bufs=6, space="PSUM") as psa, \
             tc.tile_pool(name="psv", bufs=1, space="PSUM") as psv:
            # PE prewarm: dummy f32 matmuls on a zeroed tile keep the PE-HAM
            # activity window busy during the input DMAs, so real matmuls run
            # at 2.4 GHz instead of 1.2 GHz.
            wrm = cst.tile([128, 256], F32)
            nc.gpsimd.memset(wrm[:], 0.0)
            pwarm = psa.tile([128, R], F32, tag="a")
            for _ in range(5):
                nc.tensor.matmul(pwarm[:, 0:256], wrm[:, 0:128], wrm[:],
                                 start=True, stop=True, skip_group_check=True)

            # scalar ring issues first (sync's first DMA waits on a drain),
            # so the first-matmul gate (A1) goes there
            a1t = cst.tile([128, R + HIDDEN], BF16)
            nc.scalar.dma_start(out=a1t[:], in_=inA1[:])
            a2t = cst.tile([128, HIDDEN], BF16)
            nc.sync.dma_start(out=a2t[:], in_=inA2[:])
            b1t = cst.tile([128, 2 * R], BF16)
            nc.sync.dma_start(out=b1t[:], in_=inB1[:])
            b2t = cst.tile([128, 2 * W2W], BF16)
            nc.gpsimd.dma_start(out=b2t[:], in_=inB2[:])
            auxt = cst.tile([128, 6], F32)
            nc.gpsimd.dma_start(out=auxt[:], in_=aux[:])

            xt = a1t[:, 0:R]
            w1x = a1t[:, R:R + HIDDEN]
            w1h = a2t[:]
            hst = b1t[:]
            w2b = [b2t[:, br * NCH * DIM_X:(br + 1) * NCH * DIM_X] for br in range(2)]
            cmb = [b2t[:, 2 * NCH * DIM_X + br * NCH:2 * NCH * DIM_X + (br + 1) * NCH]
                   for br in range(2)]

            # both branches accumulate into the same banks (weights pre-scaled
            # by gs/(1-gs), so the sum IS the guidance-combined result)
            pv = psv.tile([128, R], F32)
            pd = psv.tile([1, R], F32)

            # per-(chunk, branch) pieces: finer ACT/PSUM granularity keeps the
            # PE from stalling at chunk boundaries (rotating 1-bank a-tiles)
            for c in range(NCH):
                cs = bass.ts(c, 128)
                for br in range(2):
                    first = c == 0 and br == 0
                    last = c == NCH - 1 and br == 1
                    bs = bass.ts(br, R)            # branch slice in hst
                    a = psa.tile([128, R], F32, tag="a")
                    nc.tensor.matmul(a[:], w1x[:, cs], xt[:], start=True, stop=False)
                    nc.tensor.matmul(a[:], w1h[:, cs], hst[:, bs], start=False, stop=True)

                    u = actp.tile([128, R], BF16, tag="u")
                    nc.scalar.activation(u[:], a[:], AF.Tanh, bias=auxt[:, c:c + 1], scale=1.0)
                    u2 = actp.tile([128, R], BF16, tag="u2")
                    nc.vector.tensor_tensor(u2[:], u[:], u[:], op=ALU.mult)

                    nc.tensor.matmul(pv[:], w2b[br][:, cs], u[:], start=first, stop=last)
                    nc.tensor.matmul(pd[0:1, :], cmb[br][:, c:c + 1], u2[:], start=first, stop=last)

            # weights pre-scaled by gs/(1-gs)/-gs/-(1-gs): the PSUM sums ARE the
            # guidance-combined results; just add the bias terms. vout on ACT
            # and dout on DVE so the two PSUM->SBUF moves run in parallel.
            vout = outp.tile([128, R], F32)
            nc.scalar.activation(vout[:], pv[:], AF.Identity, bias=auxt[:, 4:5], scale=1.0)
            dout = outp.tile([1, R], F32)
            nc.vector.tensor_scalar(dout[:], pd[0:1, :], auxt[0:1, 5:6], None, op0=ALU.add)

            nc.sync.dma_start(out=VO[:], in_=vout[:])
            nc.scalar.dma_start(out=DO[:], in_=dout[:])
    nc.compile()
    return nc


def _get_nc():
    global _NC_CACHE
    if _NC_CACHE is None:
        _NC_CACHE = _build()
    return _NC_CACHE


def _prep_in_maps(state, h, h_null, t, guidance_scale, W1, b1, W2, b2):
    f32 = np.float32
    bf = ml_dtypes.bfloat16
    xTf = state[:, :DIM_X].T.astype(bf)                            # (128, B)
    hTf = h.T.astype(bf)
    hnTf = h_null.T.astype(bf)
    w1f = np.concatenate([W1[:DIM_X], W1[DIM_X:DIM_X + DIM_H]], axis=1).astype(bf)
    b1p = (b1.astype(f32) + t.astype(f32)[0] * W1[DIM_X + DIM_H].astype(f32))
    w2r = W2.astype(f32).reshape(NCH, 128, DIM_X).transpose(1, 0, 2).reshape(128, NCH * DIM_X)
    cvec = (W1[:DIM_X].astype(np.float64) * W2.astype(np.float64).T).sum(0)  # (512,)
    d0 = cvec.sum()
    cmatf = cvec.reshape(NCH, 128).T.astype(f32)                   # (128, NCH)
    gs = float(guidance_scale.astype(f32)[0])
    w2cf = np.concatenate([gs * w2r, (1.0 - gs) * w2r,
                           -gs * cmatf, -(1.0 - gs) * cmatf], axis=1).astype(bf)

    auxf = np.zeros((128, 6), f32)
    auxf[:, 0:4] = b1p.reshape(NCH, 128).T
    auxf[:, 4] = b2.astype(f32)
    auxf[:, 5] = d0

    w1xa = np.ascontiguousarray(w1f[:, :HIDDEN])
    w1ha = np.ascontiguousarray(w1f[:, HIDDEN:])
    in_maps = []
    for i in range(N_CORES):
        sl = slice(i * R, (i + 1) * R)
        in_maps.append({
            "inA1": np.ascontiguousarray(
                np.concatenate([xTf[:, sl], w1xa], axis=1)),
            "inA2": w1ha,
            "inB1": np.ascontiguousarray(
                np.concatenate([hTf[:, sl], hnTf[:, sl]], axis=1)),
            "inB2": w2cf,
            "aux": auxf,
        })
    return in_maps


def kernel(state, h, h_null, t, guidance_scale, W1, b1, W2, b2, _trace=False):
    nc = _get_nc()
    in_maps = _prep_in_maps(state, h, h_null, t, guidance_scale, W1, b1, W2, b2)
    res = run_bass_kernel_spmd(nc, in_maps, list(range(N_CORES)), trace=_trace)
    out = np.empty((B, DIM_X + 1), np.float32)
    for i in range(N_CORES):
        sl = slice(i * R, (i + 1) * R)
        out[sl, :DIM_X] = res.results[i]["VO"].T
        out[sl, DIM_X] = res.results[i]["DO"][0]
    if _trace:
        return out, res
    return out



# revision 2
# speedup vs baseline: 1.0783x; 1.0783x over previous
"""Trainium2 Bass kernel for CombinedVectorField (CFG vector field + exact
Jacobian-trace divergence).

Math: with u = tanh(x@W1x + h@W1h + b1'), b1' = b1 + t*W1[256],
  v(x,h)  = u @ W2 + b2
  div(x,h)= sum_k (1-u_k^2) c_k = d0 - (u*u) @ c,   c_k = sum_i W1x[i,k] W2[k,i]
Output = concat[(1-gs)*v_null + gs*v_h, (1-gs)*div_null + gs*div_h].

Sharding: pure data parallel - each of the 8 cores takes 512 batch rows
(both guidance branches), weights replicated. All tensors feature-major
(transposed) on device so every matmul contracts over the partition dim.

Schedule: inputs stream over three DMA queues (sync/scalar HWDGE + gpsimd
SWDGE) in consumption order so the z1 matmuls start as soon as the first
chunk of weights + activations lands. Both guidance branches of a hidden
chunk share one 2-bank PSUM tile ([128,1024] f32) so a single tanh
ACTIVATE covers both branches (4 wide activations instead of 8 narrow).
v-bias is fused into the vector-engine PSUM->SBUF copy (bf16 out), the
divergence bias into the scalar-engine copy.
"""
import sys

sys.path.insert(0, "/opt/trn_rl_repo")

import ml_dtypes
import numpy as np

import concourse.bass as bass
import concourse.tile as tile
from concourse import bacc, mybir
from concourse.bass_utils import run_bass_kernel_spmd
from concourse.vector_clock import ScopedClock


class _TrimTileContext(tile.TileContext):
    """TileContext with the final all-engine barrier dropped from the
    teardown and the mid barrier reduced to sem-only (no per-engine
    drains). The head drain still waits for every semaphore (incl.
    output-DMA completion) and semaphores are still cleared for the next
    execution; only the trailing barrier (nothing executes after it) is
    elided."""

    def _drain_and_barrier(self, tick_clock, wait_clock):
        drain_inst = self.nc.sync.drain()
        wait_clock.add_sem_waits(
            drain_inst.ins, ScopedClock({None: tick_clock.global_clock})
        )
        self.nc.all_engine_barrier(sem_only=True)
        popped = self.nc._tile_sem_poison_stack.pop()
        assert popped is self._sem_poison
        self.nc.clear_and_free_semaphores(list(self.sems.allocated().values()))


class _FastBacc(bacc.Bacc):
    """Bacc whose constructor-time all-engine barrier (after the const-tile
    memsets) is sem-only - the per-engine drains there cost ~1us of kernel
    head time and order nothing we rely on beyond the memsets, which the
    event-semaphore barrier already orders."""

    def all_engine_barrier(self, *, sem_only: bool = False):
        super().all_engine_barrier(sem_only=True)

F32 = mybir.dt.float32
BF16 = mybir.dt.bfloat16
AF = mybir.ActivationFunctionType
ALU = mybir.AluOpType

N_CORES = 8
B = 4096
DIM_X = 128
DIM_H = 128
HIDDEN = 512
R = B // N_CORES          # rows per core
NCH = HIDDEN // 128       # hidden chunks
W2W = NCH * DIM_X + NCH   # w2 chunks + cmat columns per branch
N_PREWARM = 4

_NC_CACHE = None


def _build():
    nc = _FastBacc("TRN2", target_bir_lowering=False, debug=False,
                   enable_asserts=False, monotonic_sem_count=0)

    # Input blobs, one queue per issuing engine, ordered so the z1 stream
    # is fed in consumption order:
    #   sync   : S1=[w1x_c0 | xT]   S2=[w1x_c123]   S3=[(1-gs)W2 | -(1-gs)c]
    #   scalar : A1=[w1h_c0 | hT]   A2=[w1h_c123]
    #   gpsimd : G0=[aux]           G1=[hnT]        G2=[gs*W2 | -gs*c]
    inS1 = nc.dram_tensor("inS1", [128, 128 + R], BF16, kind="ExternalInput")
    inS2 = nc.dram_tensor("inS2", [128, 384], BF16, kind="ExternalInput")
    inS3 = nc.dram_tensor("inS3", [128, W2W], BF16, kind="ExternalInput")
    inA1 = nc.dram_tensor("inA1", [128, 128 + R], BF16, kind="ExternalInput")
    inA2 = nc.dram_tensor("inA2", [128, 384], BF16, kind="ExternalInput")
    inG1 = nc.dram_tensor("inG1", [128, R], BF16, kind="ExternalInput")
    inG2 = nc.dram_tensor("inG2", [128, W2W], BF16, kind="ExternalInput")
    # aux cols: 0-3 b1' chunks, 4 b2, 5 d0
    aux = nc.dram_tensor("aux", [128, 6], F32, kind="ExternalInput")

    VO = nc.dram_tensor("VO", [DIM_X, R], BF16, kind="ExternalOutput")
    DO = nc.dram_tensor("DO", [1, R], F32, kind="ExternalOutput")

    with _TrimTileContext(nc) as tc:
        with tc.tile_pool(name="cst", bufs=1) as cst, \
             tc.tile_pool(name="act", bufs=4) as actp, \
             tc.tile_pool(name="out", bufs=1) as outp, \
             tc.tile_pool(name="psa", bufs=3, space="PSUM") as psa, \
             tc.tile_pool(name="psv", bufs=1, space="PSUM") as psv:
            # PE prewarm: dummy f32 matmuls on a zeroed tile keep the PE-HAM
            # activity window busy during the input DMAs, so real matmuls run
            # at 2.4 GHz instead of 1.2 GHz.
            wrm = cst.tile([128, 256], F32)
            nc.gpsimd.memset(wrm[:], 0.0)
            pwarm = psa.tile([128, 1024], F32, tag="a")
            for _ in range(N_PREWARM):
                nc.tensor.matmul(pwarm[:, 0:256], wrm[:, 0:128], wrm[:],
                                 start=True, stop=True, skip_group_check=True)

            s1t = cst.tile([128, 128 + R], BF16)
            nc.sync.dma_start(out=s1t[:], in_=inS1[:])
            s2t = cst.tile([128, 384], BF16)
            nc.sync.dma_start(out=s2t[:], in_=inS2[:])
            s3t = cst.tile([128, W2W], BF16)
            nc.sync.dma_start(out=s3t[:], in_=inS3[:])

            a1t = cst.tile([128, 128 + R], BF16)
            nc.scalar.dma_start(out=a1t[:], in_=inA1[:])
            a2t = cst.tile([128, 384], BF16)
            nc.scalar.dma_start(out=a2t[:], in_=inA2[:])

            auxt = cst.tile([128, 6], F32)
            nc.gpsimd.dma_start(out=auxt[:], in_=aux[:])
            g1t = cst.tile([128, R], BF16)
            nc.gpsimd.dma_start(out=g1t[:], in_=inG1[:])
            g2t = cst.tile([128, W2W], BF16)
            nc.gpsimd.dma_start(out=g2t[:], in_=inG2[:])

            xt = s1t[:, 128:128 + R]
            hst = [a1t[:, 128:128 + R], g1t[:]]          # hT, hnT
            w1x = [s1t[:, 0:128]] + [s2t[:, c * 128:(c + 1) * 128] for c in range(3)]
            w1h = [a1t[:, 0:128]] + [a2t[:, c * 128:(c + 1) * 128] for c in range(3)]
            # branch 0 = gs-scaled (gpsimd blob), branch 1 = (1-gs)-scaled (sync blob)
            w2b = [g2t, s3t]

            # z1 for chunk c, both branches, into one 2-bank PSUM tile
            ats = []
            for c in range(NCH):
                a = psa.tile([128, 1024], F32, tag="a")
                ats.append(a)
                for br in range(2):
                    cs = bass.ts(br, R)
                    nc.tensor.matmul(a[:, cs], w1x[c][:], xt, start=True, stop=False)
                    nc.tensor.matmul(a[:, cs], w1h[c][:], hst[br][:],
                                     start=False, stop=True)

            # one wide tanh per chunk (both branches share bias b1'_c)
            us, u2s = [], []
            for c in range(NCH):
                u = actp.tile([128, 1024], BF16, tag="u")
                nc.scalar.activation(u[:], ats[c][:], AF.Tanh,
                                     bias=auxt[:, c:c + 1], scale=1.0)
                us.append(u)
                u2 = actp.tile([128, 1024], BF16, tag="u2")
                nc.vector.tensor_tensor(u2[:], u[:], u[:], op=ALU.mult)
                u2s.append(u2)

            # weights pre-scaled by gs/(1-gs)/-gs/-(1-gs): the PSUM sums ARE the
            # guidance-combined results.
            pv = psv.tile([128, R], F32)
            pd = psv.tile([1, R], F32)
            for c in range(NCH):
                wc = slice(c * 128, (c + 1) * 128)
                cc = slice(NCH * DIM_X + c, NCH * DIM_X + c + 1)
                for br in range(2):
                    first = c == 0 and br == 0
                    cs = bass.ts(br, R)
                    nc.tensor.matmul(pv[:], w2b[br][:, wc], us[c][:, cs],
                                     start=first, stop=(c == NCH - 1 and br == 1))
                for br in range(2):
                    first = c == 0 and br == 0
                    cs = bass.ts(br, R)
                    nc.tensor.matmul(pd[0:1, :], w2b[br][:, cc], u2s[c][:, cs],
                                     start=first, stop=(c == NCH - 1 and br == 1))

            # v-bias fused into the vector copy (bf16 out); div-bias into the
            # scalar copy - the two PSUM->SBUF moves run on different engines.
            vout = outp.tile([128, R], BF16)
            nc.vector.tensor_scalar(vout[:], pv[:], auxt[:, 4:5], None, op0=ALU.add)
            dout = outp.tile([1, R], F32)
            nc.scalar.activation(dout[:], pd[0:1, :], AF.Identity,
                                 bias=auxt[0:1, 5:6], scale=1.0)

            nc.sync.dma_start(out=VO[:], in_=vout[:])
            nc.scalar.dma_start(out=DO[:], in_=dout[:])
    nc.compile()
    return nc


def _get_nc():
    global _NC_CACHE
    if _NC_CACHE is None:
        _NC_CACHE = _build()
    return _NC_CACHE


def _prep_in_maps(state, h, h_null, t, guidance_scale, W1, b1, W2, b2):
    f32 = np.float32
    bf = ml_dtypes.bfloat16
    xTf = state[:, :DIM_X].T.astype(bf)                            # (128, B)
    hTf = h.T.astype(bf)
    hnTf = h_null.T.astype(bf)
    w1xf = W1[:DIM_X].astype(bf)                                   # (128, 512)
    w1hf = W1[DIM_X:DIM_X + DIM_H].astype(bf)
    b1p = (b1.astype(f32) + t.astype(f32)[0] * W1[DIM_X + DIM_H].astype(f32))
    w2r = W2.astype(f32).reshape(NCH, 128, DIM_X).transpose(1, 0, 2).reshape(128, NCH * DIM_X)
    cvec = (W1[:DIM_X].astype(np.float64) * W2.astype(np.float64).T).sum(0)  # (512,)
    d0 = cvec.sum()
    cmatf = cvec.reshape(NCH, 128).T.astype(f32)                   # (128, NCH)
    gs = float(guidance_scale.astype(f32)[0])
    blob_gs = np.concatenate([gs * w2r, -gs * cmatf], axis=1).astype(bf)
    blob_n = np.concatenate([(1.0 - gs) * w2r, -(1.0 - gs) * cmatf], axis=1).astype(bf)

    auxf = np.zeros((128, 6), f32)
    auxf[:, 0:4] = b1p.reshape(NCH, 128).T
    auxf[:, 4] = b2.astype(f32)
    auxf[:, 5] = d0

    w1x_rest = np.ascontiguousarray(w1xf[:, 128:])
    w1h_rest = np.ascontiguousarray(w1hf[:, 128:])
    in_maps = []
    for i in range(N_CORES):
        sl = slice(i * R, (i + 1) * R)
        in_maps.append({
            "inS1": np.ascontiguousarray(
                np.concatenate([w1xf[:, 0:128], xTf[:, sl]], axis=1)),
            "inS2": w1x_rest,
            "inS3": blob_n,
            "inA1": np.ascontiguousarray(
                np.concatenate([w1hf[:, 0:128], hTf[:, sl]], axis=1)),
            "inA2": w1h_rest,
            "inG1": np.ascontiguousarray(hnTf[:, sl]),
            "inG2": blob_gs,
            "aux": auxf,
        })
    return in_maps


def kernel(state, h, h_null, t, guidance_scale, W1, b1, W2, b2, _trace=False):
    nc = _get_nc()
    in_maps = _prep_in_maps(state, h, h_null, t, guidance_scale, W1, b1, W2, b2)
    res = run_bass_kernel_spmd(nc, in_maps, list(range(N_CORES)), trace=_trace)
    out = np.empty((B, DIM_X + 1), np.float32)
    for i in range(N_CORES):
        sl = slice(i * R, (i + 1) * R)
        out[sl, :DIM_X] = res.results[i]["VO"].astype(np.float32).T
        out[sl, DIM_X] = res.results[i]["DO"][0]
    if _trace:
        return out, res
    return out


# revision 3
# speedup vs baseline: 1.0833x; 1.0046x over previous
"""Trainium2 Bass kernel for CombinedVectorField (CFG vector field + exact
Jacobian-trace divergence).

Math: with u = tanh(x@W1x + h@W1h + b1'), b1' = b1 + t*W1[256],
  v(x,h)  = u @ W2 + b2
  div(x,h)= sum_k (1-u_k^2) c_k = d0 - (u*u) @ c,   c_k = sum_i W1x[i,k] W2[k,i]
Output = concat[(1-gs)*v_null + gs*v_h, (1-gs)*div_null + gs*div_h].

Sharding: pure data parallel - each of the 8 cores takes 512 batch rows
(both guidance branches), weights replicated. All tensors feature-major
(transposed) on device so every matmul contracts over the partition dim.

Schedule: inputs stream over three DMA queues (sync/scalar HWDGE + gpsimd
SWDGE) in consumption order, split fine enough that the first z1 matmuls
start on the first ~96KB. Both guidance branches of a hidden chunk share
one 2-bank PSUM tile ([128,1024] f32) so a single tanh ACTIVATE covers
both branches; the last chunk is branch-split so the final
tanh->u^2->divergence chain is half as long. v-bias is fused into the
vector-engine PSUM->SBUF copy (bf16 out), the divergence bias into the
scalar-engine copy.

PSUM accumulation within a shared bank uses start=True only on the first
matmul that touches the bank (the start flag clears the whole bank's
has_written bits, so a later start=True would corrupt sibling column
regions; with bits cleared once, later matmuls overwrite-or-accumulate
per element correctly in any order).
"""
import sys

sys.path.insert(0, "/opt/trn_rl_repo")

import ml_dtypes
import numpy as np

import concourse.bass as bass
import concourse.tile as tile
from concourse import bacc, mybir
from concourse.bass_utils import run_bass_kernel_spmd
from concourse.vector_clock import ScopedClock


class _TrimTileContext(tile.TileContext):
    """TileContext with the final all-engine barrier dropped from the
    teardown and the mid barrier reduced to sem-only (no per-engine
    drains). The head drain still waits for every semaphore (incl.
    output-DMA completion) and semaphores are still cleared for the next
    execution; only the trailing barrier (nothing executes after it) is
    elided."""

    def _drain_and_barrier(self, tick_clock, wait_clock):
        drain_inst = self.nc.sync.drain()
        wait_clock.add_sem_waits(
            drain_inst.ins, ScopedClock({None: tick_clock.global_clock})
        )
        self.nc.all_engine_barrier(sem_only=True)
        popped = self.nc._tile_sem_poison_stack.pop()
        assert popped is self._sem_poison
        self.nc.clear_and_free_semaphores(list(self.sems.allocated().values()))


class _FastBacc(bacc.Bacc):
    """Bacc whose constructor-time all-engine barrier (after the const-tile
    memsets) is sem-only - the per-engine drains there cost ~1us of kernel
    head time and order nothing we rely on beyond the memsets, which the
    event-semaphore barrier already orders."""

    def all_engine_barrier(self, *, sem_only: bool = False):
        super().all_engine_barrier(sem_only=True)

F32 = mybir.dt.float32
BF16 = mybir.dt.bfloat16
AF = mybir.ActivationFunctionType
ALU = mybir.AluOpType

N_CORES = 8
B = 4096
DIM_X = 128
DIM_H = 128
HIDDEN = 512
R = B // N_CORES          # rows per core
HR = R // 2
NCH = HIDDEN // 128       # hidden chunks
W2W = NCH * DIM_X + NCH   # w2 chunks + cmat columns per branch
N_PREWARM = 8

_NC_CACHE = None


def _build():
    nc = _FastBacc("TRN2", target_bir_lowering=False, debug=False,
                   enable_asserts=False, monotonic_sem_count=0)

    # Input blobs, one queue per issuing engine, in consumption order:
    #   sync   : S1a=[w1x_c0|xT_lo]  S1b=[xT_hi]   SN=[(1-gs)W2 | -(1-gs)c]
    #   scalar : A1a=[w1h_c0|hT_lo]  A1b=[hT_hi]   WC1=[w1_c1]  WC23=[w1_c23]
    #   gpsimd : AUX  G1a=[hnT_lo]  G1b=[hnT_hi]  GG=[gs*W2 | -gs*c]
    inS1a = nc.dram_tensor("inS1a", [128, 128 + HR], BF16, kind="ExternalInput")
    inS1b = nc.dram_tensor("inS1b", [128, HR], BF16, kind="ExternalInput")
    inSN = nc.dram_tensor("inSN", [128, W2W], BF16, kind="ExternalInput")
    inA1a = nc.dram_tensor("inA1a", [128, 128 + HR], BF16, kind="ExternalInput")
    inA1b = nc.dram_tensor("inA1b", [128, HR], BF16, kind="ExternalInput")
    inWC1 = nc.dram_tensor("inWC1", [128, 256], BF16, kind="ExternalInput")
    inWC23 = nc.dram_tensor("inWC23", [128, 512], BF16, kind="ExternalInput")
    inG1a = nc.dram_tensor("inG1a", [128, HR], BF16, kind="ExternalInput")
    inG1b = nc.dram_tensor("inG1b", [128, HR], BF16, kind="ExternalInput")
    inGG = nc.dram_tensor("inGG", [128, W2W], BF16, kind="ExternalInput")
    # aux cols: 0-3 b1' chunks, 4 b2, 5 d0
    aux = nc.dram_tensor("aux", [128, 6], F32, kind="ExternalInput")

    VO = nc.dram_tensor("VO", [DIM_X, R], BF16, kind="ExternalOutput")
    DO = nc.dram_tensor("DO", [1, R], F32, kind="ExternalOutput")

    with _TrimTileContext(nc) as tc:
        with tc.tile_pool(name="cst", bufs=1) as cst, \
             tc.tile_pool(name="act", bufs=4) as actp, \
             tc.tile_pool(name="out", bufs=1) as outp, \
             tc.tile_pool(name="psa", bufs=3, space="PSUM") as psa, \
             tc.tile_pool(name="psv", bufs=1, space="PSUM") as psv:
            # PE prewarm: dummy bf16 matmuls on a zeroed tile keep the PE-HAM
            # activity window busy during the input DMAs so the clock ramps to
            # 2.4 GHz; fine-grained so the real stream isn't delayed much.
            wrm = cst.tile([128, 512], BF16)
            nc.gpsimd.memset(wrm[:], 0.0)
            pwarm = psa.tile([128, 1024], F32, tag="a")
            for _ in range(N_PREWARM):
                nc.tensor.matmul(pwarm[:, 0:512], wrm[:, 0:128], wrm[:],
                                 start=True, stop=True, skip_group_check=True)

            # SBUF destination tiles; xT/hT/hnT each land via two DMAs into
            # column slices of one tile (subtile deps let the lo-half matmuls
            # start before the hi half arrives).
            s1t = cst.tile([128, 128 + R], BF16)
            nc.sync.dma_start(out=s1t[:, 0:128 + HR], in_=inS1a[:])
            nc.sync.dma_start(out=s1t[:, 128 + HR:128 + R], in_=inS1b[:])
            sNt = cst.tile([128, W2W], BF16)
            nc.sync.dma_start(out=sNt[:], in_=inSN[:])

            a1t = cst.tile([128, 128 + R], BF16)
            nc.scalar.dma_start(out=a1t[:, 0:128 + HR], in_=inA1a[:])
            nc.scalar.dma_start(out=a1t[:, 128 + HR:128 + R], in_=inA1b[:])
            wc1t = cst.tile([128, 256], BF16)
            nc.scalar.dma_start(out=wc1t[:], in_=inWC1[:])
            wc23t = cst.tile([128, 512], BF16)
            nc.scalar.dma_start(out=wc23t[:], in_=inWC23[:])

            auxt = cst.tile([128, 6], F32)
            nc.gpsimd.dma_start(out=auxt[:], in_=aux[:])
            g1t = cst.tile([128, R], BF16)
            nc.gpsimd.dma_start(out=g1t[:, 0:HR], in_=inG1a[:])
            nc.gpsimd.dma_start(out=g1t[:, HR:R], in_=inG1b[:])
            gGt = cst.tile([128, W2W], BF16)
            nc.gpsimd.dma_start(out=gGt[:], in_=inGG[:])

            xt = s1t[:, 128:128 + R]
            hst = [a1t[:, 128:128 + R], g1t[:]]          # hT, hnT
            w1x = [s1t[:, 0:128], wc1t[:, 0:128],
                   wc23t[:, 0:128], wc23t[:, 256:384]]
            w1h = [a1t[:, 0:128], wc1t[:, 128:256],
                   wc23t[:, 128:256], wc23t[:, 384:512]]
            # branch 0 = gs-scaled (gpsimd blob), branch 1 = (1-gs)-scaled
            w2b = [gGt, sNt]

            # z1 chunk 0: row-split quarters so matmuls start on the lo
            # halves. Only the first matmul per PSUM bank gets start=True.
            a0 = psa.tile([128, 1024], F32, tag="a")
            for rh in range(2):
                rs = slice(rh * HR, (rh + 1) * HR)
                for br in range(2):
                    qs = slice(br * R + rh * HR, br * R + (rh + 1) * HR)
                    nc.tensor.matmul(a0[:, qs], w1x[0], xt[:, rs],
                                     start=(rh == 0), stop=False,
                                     skip_group_check=True)
                    nc.tensor.matmul(a0[:, qs], w1h[0], hst[br][:, rs],
                                     start=False, stop=(rh == 1),
                                     skip_group_check=True)
            ats = [a0]
            # chunks 1-3: full-width per branch (one bank per branch)
            for c in range(1, NCH):
                a = psa.tile([128, 1024], F32, tag="a")
                ats.append(a)
                for br in range(2):
                    cs = bass.ts(br, R)
                    nc.tensor.matmul(a[:, cs], w1x[c], xt, start=True, stop=False)
                    nc.tensor.matmul(a[:, cs], w1h[c], hst[br][:],
                                     start=False, stop=True)

            # tanh: one wide ACTIVATE for chunks 0-2; chunk 3 branch-split so
            # the final tanh->u^2->pd chain is half as long.
            us, u2s = [], []
            for c in range(NCH - 1):
                u = actp.tile([128, 1024], BF16, tag="u")
                nc.scalar.activation(u[:], ats[c][:], AF.Tanh,
                                     bias=auxt[:, c:c + 1], scale=1.0)
                us.append(u)
                u2 = actp.tile([128, 1024], BF16, tag="u2")
                nc.vector.tensor_tensor(u2[:], u[:], u[:], op=ALU.mult)
                u2s.append(u2)
            u3 = actp.tile([128, 1024], BF16, tag="u")
            u23 = actp.tile([128, 1024], BF16, tag="u2")
            for br in range(2):
                cs = bass.ts(br, R)
                nc.scalar.activation(u3[:, cs], ats[3][:, cs], AF.Tanh,
                                     bias=auxt[:, 3:4], scale=1.0)
                nc.vector.tensor_tensor(u23[:, cs], u3[:, cs], u3[:, cs],
                                        op=ALU.mult)
            us.append(u3)
            u2s.append(u23)

            # weights pre-scaled by gs/(1-gs)/-gs/-(1-gs): the PSUM sums ARE
            # the guidance-combined results. pv emitted before pd per chunk so
            # the big VO output starts as early as possible.
            pv = psv.tile([128, R], F32)
            pd = psv.tile([1, R], F32)
            for c in range(NCH):
                wc = slice(c * 128, (c + 1) * 128)
                cc = slice(NCH * DIM_X + c, NCH * DIM_X + c + 1)
                for br in range(2):
                    first = c == 0 and br == 0
                    cs = bass.ts(br, R)
                    nc.tensor.matmul(pv[:], w2b[br][:, wc], us[c][:, cs],
                                     start=first, stop=(c == NCH - 1 and br == 1))
                for br in range(2):
                    first = c == 0 and br == 0
                    cs = bass.ts(br, R)
                    nc.tensor.matmul(pd[0:1, :], w2b[br][:, cc], u2s[c][:, cs],
                                     start=first, stop=(c == NCH - 1 and br == 1))

            # v-bias fused into the vector copy (bf16 out); div-bias into the
            # scalar copy - the two PSUM->SBUF moves run on different engines.
            vout = outp.tile([128, R], BF16)
            nc.vector.tensor_scalar(vout[:], pv[:], auxt[:, 4:5], None, op0=ALU.add)
            dout = outp.tile([1, R], F32)
            nc.scalar.activation(dout[:], pd[0:1, :], AF.Identity,
                                 bias=auxt[0:1, 5:6], scale=1.0)

            nc.sync.dma_start(out=VO[:], in_=vout[:])
            nc.scalar.dma_start(out=DO[:], in_=dout[:])
    nc.compile()
    return nc


def _get_nc():
    global _NC_CACHE
    if _NC_CACHE is None:
        _NC_CACHE = _build()
    return _NC_CACHE


def _prep_in_maps(state, h, h_null, t, guidance_scale, W1, b1, W2, b2):
    f32 = np.float32
    bf = ml_dtypes.bfloat16
    xTf = state[:, :DIM_X].T.astype(bf)                            # (128, B)
    hTf = h.T.astype(bf)
    hnTf = h_null.T.astype(bf)
    w1xf = W1[:DIM_X].astype(bf)                                   # (128, 512)
    w1hf = W1[DIM_X:DIM_X + DIM_H].astype(bf)
    b1p = (b1.astype(f32) + t.astype(f32)[0] * W1[DIM_X + DIM_H].astype(f32))
    w2r = W2.astype(f32).reshape(NCH, 128, DIM_X).transpose(1, 0, 2).reshape(128, NCH * DIM_X)
    cvec = (W1[:DIM_X].astype(np.float64) * W2.astype(np.float64).T).sum(0)  # (512,)
    d0 = cvec.sum()
    cmatf = cvec.reshape(NCH, 128).T.astype(f32)                   # (128, NCH)
    gs = float(guidance_scale.astype(f32)[0])
    blob_gs = np.concatenate([gs * w2r, -gs * cmatf], axis=1).astype(bf)
    blob_n = np.concatenate([(1.0 - gs) * w2r, -(1.0 - gs) * cmatf], axis=1).astype(bf)

    auxf = np.zeros((128, 6), f32)
    auxf[:, 0:4] = b1p.reshape(NCH, 128).T
    auxf[:, 4] = b2.astype(f32)
    auxf[:, 5] = d0

    wc1 = np.ascontiguousarray(
        np.concatenate([w1xf[:, 128:256], w1hf[:, 128:256]], axis=1))
    wc23 = np.ascontiguousarray(
        np.concatenate([w1xf[:, 256:384], w1hf[:, 256:384],
                        w1xf[:, 384:512], w1hf[:, 384:512]], axis=1))
    in_maps = []
    for i in range(N_CORES):
        sl = slice(i * R, i * R + HR)
        sh = slice(i * R + HR, (i + 1) * R)
        in_maps.append({
            "inS1a": np.ascontiguousarray(
                np.concatenate([w1xf[:, 0:128], xTf[:, sl]], axis=1)),
            "inS1b": np.ascontiguousarray(xTf[:, sh]),
            "inSN": blob_n,
            "inA1a": np.ascontiguousarray(
                np.concatenate([w1hf[:, 0:128], hTf[:, sl]], axis=1)),
            "inA1b": np.ascontiguousarray(hTf[:, sh]),
            "inWC1": wc1,
            "inWC23": wc23,
            "inG1a": np.ascontiguousarray(hnTf[:, sl]),
            "inG1b": np.ascontiguousarray(hnTf[:, sh]),
            "inGG": blob_gs,
            "aux": auxf,
        })
    return in_maps


def kernel(state, h, h_null, t, guidance_scale, W1, b1, W2, b2, _trace=False):
    nc = _get_nc()
    in_maps = _prep_in_maps(state, h, h_null, t, guidance_scale, W1, b1, W2, b2)
    res = run_bass_kernel_spmd(nc, in_maps, list(range(N_CORES)), trace=_trace)
    out = np.empty((B, DIM_X + 1), np.float32)
    for i in range(N_CORES):
        sl = slice(i * R, (i + 1) * R)
        out[sl, :DIM_X] = res.results[i]["VO"].astype(np.float32).T
        out[sl, DIM_X] = res.results[i]["DO"][0]
    if _trace:
        return out, res
    return out
